# revision 27
# baseline (speedup 1.0000x reference)
"""Bass/Trainium2 kernel for BERT-style masked attention (B=1, S=4096, HID=1024, H=16).

Strategy: tensor-parallel over heads across 8 NeuronCores (2 heads/core).
Each core computes q/k/v projections for its 128 output columns from the
full (host-pretransposed) hidden states, runs masked softmax attention for
its 2 heads fully on-chip (flash-style, scores never hit DRAM), and writes
its [4096, 128] slice of the context. Host concatenates slices.

The key mask is key-only (same for every query/head), so masked key
positions are compacted away host-side: k/v projections and the attention
inner loop run only over the ~(S/2) surviving key positions.

Overlap structure: the k/v projection + v-transpose pipeline is interleaved
per 512-block with query-tile 0's attention, and each later query tile's
projection is drip-fed (one matmul per key chunk) through the preceding
tile's attention loop, so the PE and ACT engines stay dense end-to-end.
"""

import numpy as np
import ml_dtypes
from contextlib import ExitStack

import concourse.bass as bass
import concourse.tile as tile
from concourse import bacc, mybir
from concourse.bass_utils import run_bass_kernel_spmd
from concourse.masks import make_identity

f32 = mybir.dt.float32
f32r = mybir.dt.float32r
bf16 = mybir.dt.bfloat16
fp16 = mybir.dt.float16
i16 = mybir.dt.int16
AF = mybir.ActivationFunctionType
ALU = mybir.AluOpType

S = 4096
HID = 1024
D2 = 128          # per-core output columns (2 heads x 64)
NCH = HID // 128  # 8 hid chunks
NQT = S // 512    # 8 query tiles
SCALE = 64 ** -0.5
NEG = -1e30
# Schraudolph exp for DVE-offloaded chunks: bf16_bits = round(A*s + B)
EXPA = float(128 * np.log2(np.e) * SCALE)
EXPB = 16256.0 - 5.25
N_DVE = 5         # of nkca key-chunks per query tile, exp'd on DVE not ACT


def _emit(nc, tc, aps, nkb, nkca, rem):
    """nkb: # 512-wide key blocks for k/v projections (SKP = 512*nkb).
    nkca: # 128-wide key chunks the attention loop visits (<= 4*nkb).
    rem: # valid keys in the last chunk (pad rows of v1 stay zero, so
    padded keys drop out of both the context and the softmax sum)."""
    XT, XTKV, WQ, WK, WV, BQ, BK, BV, OUT = aps
    dve_kcs = set()
    if nkca >= 8:
        dve_kcs = {1 + i * nkca // N_DVE for i in range(N_DVE)}
    skp = 512 * nkb
    with ExitStack() as top:
        const = top.enter_context(tc.tile_pool(name="const", bufs=1))
        big = top.enter_context(tc.tile_pool(name="big", bufs=1))

        ident = const.tile([128, 128], f32)
        make_identity(nc, ident)

        wq = const.tile([128, NCH, 128], bf16)
        wk = const.tile([128, NCH, 128], bf16)
        wv = const.tile([128, NCH, 128], bf16)
        WQr = WQ.rearrange("(c p) d -> p c d", p=128)
        WKr = WK.rearrange("(c p) d -> p c d", p=128)
        WVr = WV.rearrange("(c p) d -> p c d", p=128)
        # weight loads are chunked and interleaved with the x-tile loads
        # below so the first projection matmuls aren't stuck behind 0.75MB
        # of weight DMA on one queue.
        nc.scalar.dma_start(out=wq[:, 0:2, :], in_=WQr[:, 0:2, :])

        bq = const.tile([128, 1], f32)
        bk = const.tile([128, 1], f32)
        bv = const.tile([128, 1], f32)
        nc.gpsimd.dma_start(out=bq, in_=BQ.unsqueeze(1))
        nc.gpsimd.dma_start(out=bk, in_=BK.unsqueeze(1))
        nc.gpsimd.dma_start(out=bv, in_=BV.unsqueeze(1))

        qT = big.tile([128, S], bf16)     # [d2, s] queries (both heads stacked)
        kT = big.tile([128, skp], bf16)   # [d2, sk] keys (compacted)
        vT = big.tile([128, skp], f32)    # [d2, sk] values (pre-transpose)
        v1 = big.tile([128, 2, nkca, 65], bf16)  # [k, head, chunk, d|1]
        ones_bf = const.tile([128, 1], bf16)
        nc.vector.memset(ones_bf, 1.0)
        if rem == 128:
            nc.vector.tensor_copy(v1[:, 0, :, 64:65], ones_bf.to_broadcast((128, nkca, 1)))
            nc.vector.tensor_copy(v1[:, 1, :, 64:65], ones_bf.to_broadcast((128, nkca, 1)))
        else:
            # last chunk: zero pad rows (v cols + ones col) so padded keys
            # contribute nothing to ctx or the softmax denominator
            for h in range(2):
                if nkca > 1:
                    nc.vector.tensor_copy(
                        v1[:, h, 0:nkca - 1, 64:65],
                        ones_bf.to_broadcast((128, nkca - 1, 1)))
                nc.vector.memset(v1[:, h, nkca - 1, :], 0.0)
                nc.vector.tensor_copy(v1[0:rem, h, nkca - 1, 64:65],
                                      ones_bf[0:rem])

        h0 = slice(0, 64)
        h1 = slice(64, 128)
        # can the next q tile's projection be burst mid-tile?
        drip = nkca >= 10

        with tc.tile_pool(name="xwkp", bufs=3) as xwkp, \
             tc.tile_pool(name="xwp", bufs=3) as xwp, \
             tc.tile_pool(name="pss", bufs=3, space="PSUM") as pss, \
             tc.tile_pool(name="psc", bufs=1, space="PSUM") as psc, \
             tc.tile_pool(name="ep", bufs=6) as ep, \
             tc.tile_pool(name="cp", bufs=4) as cp:

            # All matmul outputs (scores, projection partials, transposes)
            # share one 3-deep ring of [128,2,512] PSUM tiles (6 banks);
            # the remaining 2 banks hold the per-tile ctx accumulators.
            # Projections run as short bursts that borrow a ring slot, so
            # the score->exp->score chain gets 3 buffers of slack and the
            # PE can stay at full p-state.
            qstate = {}

            def qproj_dma(qt):
                qsl = slice(qt * 512, (qt + 1) * 512)
                xw = xwp.tile([128, NCH, 512], bf16, tag="xw", name=f"xw{qt}")
                engs = [nc.gpsimd, nc.gpsimd, nc.sync, nc.gpsimd,
                        nc.gpsimd, nc.gpsimd, nc.sync, nc.sync]
                for c in range(NCH):
                    engs[c].dma_start(
                        out=xw[:, c, :],
                        in_=XT[c * 128:(c + 1) * 128, qsl])
                qstate[qt] = xw

            kvx = {}

            def prefetch_kv(kb):
                # round-robin the kv x-chunks over two queues well before
                # k_block(kb) needs them
                if kb >= nkb or kb in kvx:
                    return
                sl = slice(kb * 512, (kb + 1) * 512)
                xw = xwkp.tile([128, NCH, 512], bf16, tag="xwk",
                               name=f"xwk{kb}")
                engs = [nc.sync, nc.gpsimd] * 4
                for c in range(NCH):
                    engs[c].dma_start(
                        out=xw[:, c, :],
                        in_=XTKV[c * 128:(c + 1) * 128, sl])
                kvx[kb] = xw

            qdone = set()

            def qproj_burst(qt):
                qdone.add(qt)
                xw = qstate[qt]
                rs = pss.tile([128, 2, 512], f32, tag="ss", name=f"rq{qt}")
                for c in range(NCH):
                    nc.tensor.matmul(rs[:, 0, :], wq[:, c, :], xw[:, c, :],
                                     start=(c == 0), stop=(c == NCH - 1),
                                     skip_group_check=True)
                qsl = slice(qt * 512, (qt + 1) * 512)
                nc.vector.tensor_scalar_add(qT[:, qsl], rs[:, 0, :], bq)

            def k_block(kb, xw=None):
                sl = slice(kb * 512, (kb + 1) * 512)
                if xw is None:
                    xw = kvx.pop(kb)
                pk = pss.tile([128, 2, 512], f32, tag="ss", name=f"rk{kb}")
                for c in range(NCH):
                    nc.tensor.matmul(pk[:, 0, :], wk[:, c, :], xw[:, c, :],
                                     start=(c == 0), stop=(c == NCH - 1))
                nc.vector.tensor_scalar_add(kT[:, sl], pk[:, 0, :], bk)
                return xw

            def v_block(kb, xw):
                sl = slice(kb * 512, (kb + 1) * 512)
                pv = pss.tile([128, 2, 512], f32, tag="ss", name=f"rv{kb}")
                for c in range(NCH):
                    nc.tensor.matmul(pv[:, 0, :], wv[:, c, :], xw[:, c, :],
                                     start=(c == 0), stop=(c == NCH - 1))
                nc.vector.tensor_scalar_add(vT[:, sl], pv[:, 0, :], bv)

            def vt_chunk(kc):
                pt = pss.tile([128, 2, 512], f32, tag="ss", name=f"vt{kc}")
                nc.tensor.transpose(pt[:, 0, 0:128],
                                    vT[:, kc * 128:(kc + 1) * 128], ident)
                rows = 128 if kc < nkca - 1 else rem
                nc.vector.tensor_copy(v1[0:rows, 0, kc, 0:64],
                                      pt[0:rows, 0, 0:64])
                nc.vector.tensor_copy(v1[0:rows, 1, kc, 0:64],
                                      pt[0:rows, 0, 64:128])

            att = {}

            def att_begin(qt):
                if qt not in qstate:
                    qproj_dma(qt)
                if qt not in qdone:
                    qproj_burst(qt)
                pc0 = psc.tile([65, 512], f32, tag="pc0", name=f"pc0_{qt}")
                pc1 = psc.tile([65, 512], f32, tag="pc1", name=f"pc1_{qt}")
                att[qt] = [pc0, pc1, []]

            def att_chunk(qt, kc):
                st = att[qt]
                qsl = slice(qt * 512, (qt + 1) * 512)
                ksl = slice(kc * 128, (kc + 1) * 128)
                ss = pss.tile([128, 2, 512], f32, tag="ss", name=f"ss{qt}_{kc}")
                nc.tensor.matmul(ss[:, 0, :], kT[h0, ksl], qT[h0, qsl],
                                 start=True, stop=True)
                nc.tensor.matmul(ss[:, 1, :], kT[h1, ksl], qT[h1, qsl],
                                 start=True, stop=True)
                if kc in dve_kcs:
                    eti = ep.tile([128, 2, 512], i16, tag="eti",
                                  name=f"eti{qt}_{kc}")
                    nc.vector.tensor_scalar(
                        out=eti, in0=ss, scalar1=EXPA, scalar2=EXPB,
                        op0=ALU.mult, op1=ALU.add)
                    et = eti.bitcast(bf16)
                else:
                    et = ep.tile([128, 2, 512], bf16, tag="et",
                                 name=f"et{qt}_{kc}")
                    nc.scalar.activation(et, ss, AF.Exp, scale=SCALE)
                st[2].append((kc, et))
                if len(st[2]) > 3:
                    # run ctx three chunks behind exp so the exp engines have
                    # three full periods of latency slack before PE needs et
                    pkc, pet = st[2].pop(0)
                    nc.tensor.matmul(st[0], v1[:, 0, pkc, :], pet[:, 0, :],
                                     start=(pkc == 0), stop=False)
                    nc.tensor.matmul(st[1], v1[:, 1, pkc, :], pet[:, 1, :],
                                     start=(pkc == 0), stop=False)
                if drip and qt + 1 < NQT:
                    # prefetch next tile's x at kc 8, project it in one
                    # burst at kc 12 (borrows one ss-ring slot for ~2 chunks)
                    if kc == nkca - 8:
                        qproj_dma(qt + 1)
                    elif kc == nkca - 4:
                        qproj_burst(qt + 1)

            def att_flush(qt):
                pc0, pc1, pend = att[qt]
                while pend:
                    pkc, pet = pend.pop(0)
                    last = not pend
                    nc.tensor.matmul(pc0, v1[:, 0, pkc, :], pet[:, 0, :],
                                     start=(pkc == 0), stop=last)
                    nc.tensor.matmul(pc1, v1[:, 1, pkc, :], pet[:, 1, :],
                                     start=(pkc == 0), stop=last)

            def att_drain(qt):
                # free the PSUM accumulators right away: one copy per head
                # grabs ctx rows 0..63 plus the l row (64); host divides.
                # h0 via DVE, h1 via ACT so neither engine serializes both.
                for h in range(2):
                    pc = att[qt][h]
                    lc = cp.tile([65, 512], f32, tag="lc",
                                 name=f"lc{qt}_{h}")
                    if h == 0:
                        nc.vector.tensor_copy(lc, pc)
                    else:
                        nc.scalar.activation(lc, pc, AF.Copy)
                    nc.sync.dma_start(
                        out=OUT[h, :, qt * 512:(qt + 1) * 512], in_=lc)

            # ---- qt 0: interleaved with k/v projection blocks.
            # Critical-path DMAs first: qt0's x chunks and kv block 0's x
            # chunks alternate on the queue so both projection chains start
            # as early as possible.
            qsl0 = slice(0, 512)
            xw0 = xwp.tile([128, NCH, 512], bf16, tag="xw", name="xw0")
            xwk0 = xwkp.tile([128, NCH, 512], bf16, tag="xwk", name="xwk0")

            def ldx(eng, c):
                eng.dma_start(out=xw0[:, c, :],
                              in_=XT[c * 128:(c + 1) * 128, qsl0])

            # startup choreography: qproj inputs (wq chunks + xw0 chunks)
            # arrive in consumption order across 3 queues, then k-proj
            # inputs, so the PE rarely waits on HBM during warmup.
            ldx(nc.gpsimd, 0)
            nc.sync.dma_start(out=xwk0[:, 0, :], in_=XTKV[0:128, 0:512])
            ldx(nc.scalar, 1)
            ldx(nc.sync, 2)
            ldx(nc.gpsimd, 3)
            nc.scalar.dma_start(out=wq[:, 2:5, :], in_=WQr[:, 2:5, :])
            ldx(nc.sync, 5)
            ldx(nc.scalar, 4)
            ldx(nc.gpsimd, 6)
            nc.scalar.dma_start(out=wq[:, 5:8, :], in_=WQr[:, 5:8, :])
            ldx(nc.scalar, 7)
            nc.scalar.dma_start(out=wk[:, 0:2, :], in_=WKr[:, 0:2, :])
            for c in range(1, NCH):
                nc.sync.dma_start(
                    out=xwk0[:, c, :],
                    in_=XTKV[c * 128:(c + 1) * 128, 0:512])
            nc.scalar.dma_start(out=wk[:, 2:8, :], in_=WKr[:, 2:8, :])
            nc.sync.dma_start(out=wv[:, 0:4, :], in_=WVr[:, 0:4, :])
            nc.scalar.dma_start(out=wv[:, 4:8, :], in_=WVr[:, 4:8, :])
            prefetch_kv(1)
            qstate[0] = xw0
            qproj_burst(0)
            pc0_t = psc.tile([65, 512], f32, tag="pc0", name="pc0_0")
            pc1_t = psc.tile([65, 512], f32, tag="pc1", name="pc1_0")
            att[0] = [pc0_t, pc1_t, []]
            for kb in range(nkb):
                prefetch_kv(kb + 2)
                xw = k_block(kb, xw=xwk0 if kb == 0 else None)
                if kb * 4 < nkca:
                    att_chunk(0, kb * 4)  # ctx inside uses v1[kc-1] (ready)
                v_block(kb, xw)
                for j in range(4):
                    kc = kb * 4 + j
                    if kc >= nkca:
                        break
                    vt_chunk(kc)
                    if j > 0:
                        att_chunk(0, kc)
            att_flush(0)
            # ---- remaining q tiles; the previous tile's PSUM drain is
            # issued after this tile's first exps so the exp engines aren't
            # stuck behind the drain copies at the tile boundary.
            for qt in range(1, NQT):
                att_begin(qt)
                for kc in range(nkca):
                    att_chunk(qt, kc)
                    if kc == 1:
                        att_drain(qt - 1)
                att_flush(qt)
            att_drain(NQT - 1)


_NC = {}


def _build(nkb, nkca, rem):
    key = (nkb, nkca, rem)
    if key in _NC:
        return _NC[key]
    nc = bacc.Bacc("TRN2", target_bir_lowering=False, debug=False)
    skp = 512 * nkb
    XT = nc.dram_tensor("XT", [HID, S], bf16, kind="ExternalInput").ap()
    XTKV = nc.dram_tensor("XTKV", [HID, skp], bf16, kind="ExternalInput").ap()
    WQ = nc.dram_tensor("WQ", [HID, D2], bf16, kind="ExternalInput").ap()
    WK = nc.dram_tensor("WK", [HID, D2], bf16, kind="ExternalInput").ap()
    WV = nc.dram_tensor("WV", [HID, D2], bf16, kind="ExternalInput").ap()
    BQ = nc.dram_tensor("BQ", [D2], f32, kind="ExternalInput").ap()
    BK = nc.dram_tensor("BK", [D2], f32, kind="ExternalInput").ap()
    BV = nc.dram_tensor("BV", [D2], f32, kind="ExternalInput").ap()
    OUT = nc.dram_tensor("OUT", [2, 65, S], f32, kind="ExternalOutput").ap()
    with tile.TileContext(nc) as tc:
        _emit(nc, tc, (XT, XTKV, WQ, WK, WV, BQ, BK, BV, OUT),
              nkb, nkca, rem)
    nc.compile()
    _NC[key] = nc
    return nc


def make_in_maps(hidden_states, attention_mask, Wq, bq, Wk, bk, Wv, bv):
    bfl = ml_dtypes.bfloat16
    x = np.asarray(hidden_states, dtype=np.float32).reshape(S, HID)
    xT = np.ascontiguousarray(x.T.astype(bfl))
    mask = np.asarray(attention_mask).reshape(S).astype(bool)
    idx = np.nonzero(mask)[0]
    m = len(idx)
    nkca = max(1, (m + 127) // 128)
    nkb = max(1, (nkca * 128 + 511) // 512)
    skp = nkb * 512
    # pad with position 0 (values are finite; pad slots masked to -inf below)
    idx_p = np.zeros(skp, np.int64)
    idx_p[:m] = idx
    xTkv = np.ascontiguousarray(xT[:, idx_p])
    rem = m - 128 * (nkca - 1)
    Wq = np.asarray(Wq, np.float32).astype(bfl)
    Wk = np.asarray(Wk, np.float32).astype(bfl)
    Wv = np.asarray(Wv, np.float32).astype(bfl)
    bq = np.asarray(bq, np.float32)
    bk = np.asarray(bk, np.float32)
    bv = np.asarray(bv, np.float32)
    in_maps = []
    for c in range(8):
        sl = slice(D2 * c, D2 * (c + 1))
        in_maps.append({
            "XT": xT, "XTKV": xTkv,
            "WQ": np.ascontiguousarray(Wq[:, sl]),
            "WK": np.ascontiguousarray(Wk[:, sl]),
            "WV": np.ascontiguousarray(Wv[:, sl]),
            "BQ": np.ascontiguousarray(bq[sl]),
            "BK": np.ascontiguousarray(bk[sl]),
            "BV": np.ascontiguousarray(bv[sl]),
        })
    return in_maps, nkb, nkca, rem


def kernel(hidden_states, attention_mask, Wq, bq, Wk, bk, Wv, bv):
    in_maps, nkb, nkca, rem = make_in_maps(
        hidden_states, attention_mask, Wq, bq, Wk, bk, Wv, bv)
    nc = _build(nkb, nkca, rem)
    res = run_bass_kernel_spmd(nc, in_maps, list(range(8)))
    # OUT[c]: [2, 65, S]; rows 0..63 = unnormalized ctx (transposed),
    # row 64 = softmax denominator l. Normalize + untranspose on host.
    outs = []
    for c in range(8):
        o = res.results[c]["OUT"].astype(np.float64)
        outs.append(o[:, 0:64, :] / o[:, 64:65, :])
    outT = np.concatenate(outs, axis=0).reshape(HID, S)
    return (np.ascontiguousarray(outT.T.astype(np.float32)).reshape(1, S, HID),)



# revision 35
# speedup vs baseline: 1.1457x; 1.1457x over previous
"""Bass/Trainium2 kernel for BERT-style masked attention (B=1, S=4096, HID=1024, H=16).

Strategy: tensor-parallel over heads across 8 NeuronCores (2 heads/core).
Each core computes q/k/v projections for its 128 output columns from the
full (host-pretransposed) hidden states, runs masked softmax attention for
its 2 heads fully on-chip (flash-style, scores never hit DRAM), and writes
its [4096, 128] slice of the context. Host concatenates slices.

The key mask is key-only (same for every query/head), so masked key
positions are compacted away host-side: k/v projections and the attention
inner loop run only over the ~(S/2) surviving key positions.

Overlap structure: the k/v projection + v-transpose pipeline is interleaved
per 512-block with query-tile 0's attention, and each later query tile's
projection is drip-fed (one matmul per key chunk) through the preceding
tile's attention loop, so the PE and ACT engines stay dense end-to-end.
"""

import numpy as np
import ml_dtypes
from contextlib import ExitStack

import concourse.bass as bass
import concourse.tile as tile
from concourse import bacc, mybir
from concourse.bass_utils import run_bass_kernel_spmd
from concourse.masks import make_identity

f32 = mybir.dt.float32
f32r = mybir.dt.float32r
bf16 = mybir.dt.bfloat16
fp16 = mybir.dt.float16
i16 = mybir.dt.int16
AF = mybir.ActivationFunctionType
ALU = mybir.AluOpType

S = 4096
HID = 1024
D2 = 128          # per-core output columns (2 heads x 64)
NCH = HID // 128  # 8 hid chunks
NQT = S // 512    # 8 query tiles
SCALE = 64 ** -0.5
NEG = -1e30
# Schraudolph exp for DVE-offloaded chunks: bf16_bits = round(A*s + B)
EXPA = float(128 * np.log2(np.e) * SCALE)
EXPB = 16256.0 - 5.25
N_DVE = 5         # of nkca key-chunks per query tile, exp'd on DVE not ACT


def _emit(nc, tc, aps, nkb, nkca, rem):
    """nkb: # 512-wide key blocks for k/v projections (SKP = 512*nkb).
    nkca: # 128-wide key chunks the attention loop visits (<= 4*nkb).
    rem: # valid keys in the last chunk (pad rows of v1 stay zero, so
    padded keys drop out of both the context and the softmax sum)."""
    XT, XTKV, WQ, WK, WV, BQ, BK, BV, OUT = aps
    dve_kcs = set()
    if nkca >= 16:
        # odd chunks 5..13: late enough that the DVE queue (drain copies at
        # kc 1-2) is clear, spaced 2 apart so DVE keeps up with the PE
        dve_kcs = {nkca - 11 + 2 * i for i in range(N_DVE)}
    elif nkca >= 8:
        dve_kcs = {1 + i * nkca // N_DVE for i in range(N_DVE)}
    skp = 512 * nkb
    with ExitStack() as top:
        const = top.enter_context(tc.tile_pool(name="const", bufs=1))
        big = top.enter_context(tc.tile_pool(name="big", bufs=1))

        ident = const.tile([128, 128], f32)
        make_identity(nc, ident)

        wq = const.tile([128, NCH, 128], bf16)
        wk = const.tile([128, NCH, 128], bf16)
        wv = const.tile([128, NCH, 128], bf16)
        WQr = WQ.rearrange("(c p) d -> p c d", p=128)
        WKr = WK.rearrange("(c p) d -> p c d", p=128)
        WVr = WV.rearrange("(c p) d -> p c d", p=128)
        # weight loads are chunked and interleaved with the x-tile loads
        # below so the first projection matmuls aren't stuck behind 0.75MB
        # of weight DMA on one queue.
        nc.scalar.dma_start(out=wq[:, 0:2, :], in_=WQr[:, 0:2, :])

        bq = const.tile([128, 1], f32)
        bk = const.tile([128, 1], f32)
        bv = const.tile([128, 1], f32)
        nc.gpsimd.dma_start(out=bq, in_=BQ.unsqueeze(1))
        nc.gpsimd.dma_start(out=bk, in_=BK.unsqueeze(1))
        nc.gpsimd.dma_start(out=bv, in_=BV.unsqueeze(1))

        qT = big.tile([128, S], bf16)     # [d2, s] queries (both heads stacked)
        kT = big.tile([128, skp], bf16)   # [d2, sk] keys (compacted)
        vT = big.tile([128, skp], f32)    # [d2, sk] values (pre-transpose)
        v1 = big.tile([128, 2, nkca, 65], bf16)  # [k, head, chunk, d|1]
        ones_bf = const.tile([128, 1], bf16)
        nc.vector.memset(ones_bf, 1.0)
        if rem == 128:
            nc.vector.tensor_copy(v1[:, 0, :, 64:65], ones_bf.to_broadcast((128, nkca, 1)))
            nc.vector.tensor_copy(v1[:, 1, :, 64:65], ones_bf.to_broadcast((128, nkca, 1)))
        else:
            # last chunk: zero pad rows (v cols + ones col) so padded keys
            # contribute nothing to ctx or the softmax denominator
            for h in range(2):
                if nkca > 1:
                    nc.vector.tensor_copy(
                        v1[:, h, 0:nkca - 1, 64:65],
                        ones_bf.to_broadcast((128, nkca - 1, 1)))
                nc.vector.memset(v1[:, h, nkca - 1, :], 0.0)
                nc.vector.tensor_copy(v1[0:rem, h, nkca - 1, 64:65],
                                      ones_bf[0:rem])

        h0 = slice(0, 64)
        h1 = slice(64, 128)
        # can the next q tile's projection be dripped through kc 4..14?
        drip = nkca >= 13

        with tc.tile_pool(name="xwkp", bufs=3) as xwkp, \
             tc.tile_pool(name="xwp", bufs=3) as xwp, \
             tc.tile_pool(name="pkv", bufs=1, space="PSUM") as pkv, \
             tc.tile_pool(name="ppq", bufs=1, space="PSUM") as ppq, \
             tc.tile_pool(name="pss", bufs=2, space="PSUM") as pss, \
             tc.tile_pool(name="psc", bufs=1, space="PSUM") as psc, \
             tc.tile_pool(name="ep", bufs=6) as ep, \
             tc.tile_pool(name="cp", bufs=4) as cp:

            qstate = {}

            def qproj_dma(qt):
                qsl = slice(qt * 512, (qt + 1) * 512)
                xw = xwp.tile([128, NCH, 512], bf16, tag="xw", name=f"xw{qt}")
                engs = [nc.gpsimd, nc.gpsimd, nc.sync, nc.gpsimd,
                        nc.gpsimd, nc.gpsimd, nc.sync, nc.sync]
                for c in range(NCH):
                    engs[c].dma_start(
                        out=xw[:, c, :],
                        in_=XT[c * 128:(c + 1) * 128, qsl])
                pq = ppq.tile([128, 512], f32, tag="pqq", name=f"pq{qt}")
                qstate[qt] = (xw, pq)

            kvx = {}

            def prefetch_kv(kb):
                # round-robin the kv x-chunks over two queues well before
                # k_block(kb) needs them
                if kb >= nkb or kb in kvx:
                    return
                sl = slice(kb * 512, (kb + 1) * 512)
                xw = xwkp.tile([128, NCH, 512], bf16, tag="xwk",
                               name=f"xwk{kb}")
                engs = [nc.sync, nc.gpsimd] * 4
                for c in range(NCH):
                    engs[c].dma_start(
                        out=xw[:, c, :],
                        in_=XTKV[c * 128:(c + 1) * 128, sl])
                kvx[kb] = xw

            qdone = set()

            def qproj_mm(qt, c):
                xw, pq = qstate[qt]
                nc.tensor.matmul(pq, wq[:, c, :], xw[:, c, :],
                                 start=(c == 0), stop=(c == NCH - 1),
                                 skip_group_check=True)

            def qproj_bias(qt):
                qdone.add(qt)
                _, pq = qstate[qt]
                qsl = slice(qt * 512, (qt + 1) * 512)
                nc.vector.tensor_scalar_add(qT[:, qsl], pq, bq)

            def k_block(kb, xw=None):
                sl = slice(kb * 512, (kb + 1) * 512)
                if xw is None:
                    xw = kvx.pop(kb)
                pk = pkv.tile([128, 512], f32, tag="pkv", name=f"pk{kb}")
                for c in range(NCH):
                    nc.tensor.matmul(pk, wk[:, c, :], xw[:, c, :],
                                     start=(c == 0), stop=(c == NCH - 1))
                nc.vector.tensor_scalar_add(kT[:, sl], pk, bk)
                return xw

            def v_block(kb, xw):
                sl = slice(kb * 512, (kb + 1) * 512)
                pv = pkv.tile([128, 512], f32, tag="pkv", name=f"pv{kb}")
                for c in range(NCH):
                    nc.tensor.matmul(pv, wv[:, c, :], xw[:, c, :],
                                     start=(c == 0), stop=(c == NCH - 1))
                nc.vector.tensor_scalar_add(vT[:, sl], pv, bv)

            def vt_chunk(kc):
                pt = pss.tile([128, 2, 512], f32, tag="ss", name=f"vt{kc}")
                nc.tensor.transpose(pt[:, 0, 0:128],
                                    vT[:, kc * 128:(kc + 1) * 128], ident)
                rows = 128 if kc < nkca - 1 else rem
                nc.vector.tensor_copy(v1[0:rows, 0, kc, 0:64],
                                      pt[0:rows, 0, 0:64])
                nc.vector.tensor_copy(v1[0:rows, 1, kc, 0:64],
                                      pt[0:rows, 0, 64:128])

            att = {}

            def att_begin(qt):
                if qt not in qstate:
                    qproj_dma(qt)
                if qt not in qdone:
                    for c in range(NCH):
                        qproj_mm(qt, c)
                    qproj_bias(qt)
                pc0 = psc.tile([65, 512], f32, tag="pc0", name=f"pc0_{qt}")
                pc1 = psc.tile([65, 512], f32, tag="pc1", name=f"pc1_{qt}")
                att[qt] = [pc0, pc1, []]

            def att_chunk(qt, kc):
                st = att[qt]
                qsl = slice(qt * 512, (qt + 1) * 512)
                ksl = slice(kc * 128, (kc + 1) * 128)
                ss = pss.tile([128, 2, 512], f32, tag="ss", name=f"ss{qt}_{kc}")
                nc.tensor.matmul(ss[:, 0, :], kT[h0, ksl], qT[h0, qsl],
                                 start=True, stop=True)
                nc.tensor.matmul(ss[:, 1, :], kT[h1, ksl], qT[h1, qsl],
                                 start=True, stop=True)
                if kc in dve_kcs:
                    eti = ep.tile([128, 2, 512], i16, tag="eti",
                                  name=f"eti{qt}_{kc}")
                    nc.vector.tensor_scalar(
                        out=eti, in0=ss, scalar1=EXPA, scalar2=EXPB,
                        op0=ALU.mult, op1=ALU.add)
                    et = eti.bitcast(bf16)
                else:
                    et = ep.tile([128, 2, 512], bf16, tag="et",
                                 name=f"et{qt}_{kc}")
                    nc.scalar.activation(et, ss, AF.Exp, scale=SCALE)
                st[2].append((kc, et))
                if len(st[2]) > 3:
                    # run ctx three chunks behind exp so the exp engines have
                    # three full periods of latency slack before PE needs et
                    pkc, pet = st[2].pop(0)
                    nc.tensor.matmul(st[0], v1[:, 0, pkc, :], pet[:, 0, :],
                                     start=(pkc == 0), stop=False)
                    nc.tensor.matmul(st[1], v1[:, 1, pkc, :], pet[:, 1, :],
                                     start=(pkc == 0), stop=False)
                if drip and qt + 1 < NQT:
                    # drip next tile's projection through kc 4..11 and its
                    # bias-add at kc 14, keeping the DVE queue clear around
                    # the DVE-exp chunks so their latency stays low
                    if kc == 0:
                        qproj_dma(qt + 1)
                    elif 4 <= kc <= 11:
                        qproj_mm(qt + 1, kc - 4)
                    elif kc == 14:
                        qproj_bias(qt + 1)

            def att_flush(qt):
                pc0, pc1, pend = att[qt]
                while pend:
                    pkc, pet = pend.pop(0)
                    last = not pend
                    nc.tensor.matmul(pc0, v1[:, 0, pkc, :], pet[:, 0, :],
                                     start=(pkc == 0), stop=last)
                    nc.tensor.matmul(pc1, v1[:, 1, pkc, :], pet[:, 1, :],
                                     start=(pkc == 0), stop=last)

            def att_drain(qt):
                # free the PSUM accumulators right away: one copy per head
                # grabs ctx rows 0..63 plus the l row (64); host divides.
                # h0 via DVE, h1 via ACT so neither engine serializes both.
                for h in range(2):
                    pc = att[qt][h]
                    lc = cp.tile([65, 512], f32, tag="lc",
                                 name=f"lc{qt}_{h}")
                    if h == 0:
                        nc.vector.tensor_copy(lc, pc)
                    else:
                        nc.scalar.activation(lc, pc, AF.Copy)
                    nc.sync.dma_start(
                        out=OUT[h, :, qt * 512:(qt + 1) * 512], in_=lc)

            # ---- qt 0: interleaved with k/v projection blocks.
            # Critical-path DMAs first: qt0's x chunks and kv block 0's x
            # chunks alternate on the queue so both projection chains start
            # as early as possible.
            qsl0 = slice(0, 512)
            xw0 = xwp.tile([128, NCH, 512], bf16, tag="xw", name="xw0")
            xwk0 = xwkp.tile([128, NCH, 512], bf16, tag="xwk", name="xwk0")

            def ldx(eng, c):
                eng.dma_start(out=xw0[:, c, :],
                              in_=XT[c * 128:(c + 1) * 128, qsl0])

            # startup choreography: qproj inputs (wq chunks + xw0 chunks)
            # arrive in consumption order across 3 queues, then k-proj
            # inputs, so the PE rarely waits on HBM during warmup.
            ldx(nc.gpsimd, 0)
            nc.sync.dma_start(out=xwk0[:, 0, :], in_=XTKV[0:128, 0:512])
            ldx(nc.scalar, 1)
            ldx(nc.sync, 2)
            ldx(nc.gpsimd, 3)
            nc.scalar.dma_start(out=wq[:, 2:5, :], in_=WQr[:, 2:5, :])
            ldx(nc.sync, 5)
            ldx(nc.scalar, 4)
            ldx(nc.gpsimd, 6)
            nc.scalar.dma_start(out=wq[:, 5:8, :], in_=WQr[:, 5:8, :])
            ldx(nc.scalar, 7)
            nc.scalar.dma_start(out=wk[:, 0:2, :], in_=WKr[:, 0:2, :])
            for c in range(1, NCH):
                nc.sync.dma_start(
                    out=xwk0[:, c, :],
                    in_=XTKV[c * 128:(c + 1) * 128, 0:512])
            nc.scalar.dma_start(out=wk[:, 2:8, :], in_=WKr[:, 2:8, :])
            nc.sync.dma_start(out=wv[:, 0:4, :], in_=WVr[:, 0:4, :])
            nc.scalar.dma_start(out=wv[:, 4:8, :], in_=WVr[:, 4:8, :])
            prefetch_kv(1)
            pq0 = ppq.tile([128, 512], f32, tag="pqq", name="pq0")
            qstate[0] = (xw0, pq0)
            for c in range(NCH):
                qproj_mm(0, c)
            qproj_bias(0)
            pc0_t = psc.tile([65, 512], f32, tag="pc0", name="pc0_0")
            pc1_t = psc.tile([65, 512], f32, tag="pc1", name="pc1_0")
            att[0] = [pc0_t, pc1_t, []]
            for kb in range(nkb):
                prefetch_kv(kb + 2)
                xw = k_block(kb, xw=xwk0 if kb == 0 else None)
                if kb * 4 < nkca:
                    att_chunk(0, kb * 4)  # ctx inside uses v1[kc-1] (ready)
                v_block(kb, xw)
                for j in range(4):
                    kc = kb * 4 + j
                    if kc >= nkca:
                        break
                    vt_chunk(kc)
                    if j > 0:
                        att_chunk(0, kc)
            att_flush(0)
            # ---- remaining q tiles; the previous tile's PSUM drain is
            # issued after this tile's first exps so the exp engines aren't
            # stuck behind the drain copies at the tile boundary.
            for qt in range(1, NQT):
                att_begin(qt)
                for kc in range(nkca):
                    att_chunk(qt, kc)
                    if kc == 1:
                        att_drain(qt - 1)
                att_flush(qt)
            att_drain(NQT - 1)


_NC = {}


def _build(nkb, nkca, rem):
    key = (nkb, nkca, rem)
    if key in _NC:
        return _NC[key]
    nc = bacc.Bacc("TRN2", target_bir_lowering=False, debug=False)
    skp = 512 * nkb
    XT = nc.dram_tensor("XT", [HID, S], bf16, kind="ExternalInput").ap()
    XTKV = nc.dram_tensor("XTKV", [HID, skp], bf16, kind="ExternalInput").ap()
    WQ = nc.dram_tensor("WQ", [HID, D2], bf16, kind="ExternalInput").ap()
    WK = nc.dram_tensor("WK", [HID, D2], bf16, kind="ExternalInput").ap()
    WV = nc.dram_tensor("WV", [HID, D2], bf16, kind="ExternalInput").ap()
    BQ = nc.dram_tensor("BQ", [D2], f32, kind="ExternalInput").ap()
    BK = nc.dram_tensor("BK", [D2], f32, kind="ExternalInput").ap()
    BV = nc.dram_tensor("BV", [D2], f32, kind="ExternalInput").ap()
    OUT = nc.dram_tensor("OUT", [2, 65, S], f32, kind="ExternalOutput").ap()
    with tile.TileContext(nc) as tc:
        _emit(nc, tc, (XT, XTKV, WQ, WK, WV, BQ, BK, BV, OUT),
              nkb, nkca, rem)
    nc.compile()
    _NC[key] = nc
    return nc


def make_in_maps(hidden_states, attention_mask, Wq, bq, Wk, bk, Wv, bv):
    bfl = ml_dtypes.bfloat16
    x = np.asarray(hidden_states, dtype=np.float32).reshape(S, HID)
    xT = np.ascontiguousarray(x.T.astype(bfl))
    mask = np.asarray(attention_mask).reshape(S).astype(bool)
    idx = np.nonzero(mask)[0]
    m = len(idx)
    nkca = max(1, (m + 127) // 128)
    nkb = max(1, (nkca * 128 + 511) // 512)
    skp = nkb * 512
    # pad with position 0 (values are finite; pad slots masked to -inf below)
    idx_p = np.zeros(skp, np.int64)
    idx_p[:m] = idx
    xTkv = np.ascontiguousarray(xT[:, idx_p])
    rem = m - 128 * (nkca - 1)
    Wq = np.asarray(Wq, np.float32).astype(bfl)
    Wk = np.asarray(Wk, np.float32).astype(bfl)
    Wv = np.asarray(Wv, np.float32).astype(bfl)
    bq = np.asarray(bq, np.float32)
    bk = np.asarray(bk, np.float32)
    bv = np.asarray(bv, np.float32)
    in_maps = []
    for c in range(8):
        sl = slice(D2 * c, D2 * (c + 1))
        in_maps.append({
            "XT": xT, "XTKV": xTkv,
            "WQ": np.ascontiguousarray(Wq[:, sl]),
            "WK": np.ascontiguousarray(Wk[:, sl]),
            "WV": np.ascontiguousarray(Wv[:, sl]),
            "BQ": np.ascontiguousarray(bq[sl]),
            "BK": np.ascontiguousarray(bk[sl]),
            "BV": np.ascontiguousarray(bv[sl]),
        })
    return in_maps, nkb, nkca, rem


def kernel(hidden_states, attention_mask, Wq, bq, Wk, bk, Wv, bv):
    in_maps, nkb, nkca, rem = make_in_maps(
        hidden_states, attention_mask, Wq, bq, Wk, bk, Wv, bv)
    nc = _build(nkb, nkca, rem)
    res = run_bass_kernel_spmd(nc, in_maps, list(range(8)))
    # OUT[c]: [2, 65, S]; rows 0..63 = unnormalized ctx (transposed),
    # row 64 = softmax denominator l. Normalize + untranspose on host.
    outs = []
    for c in range(8):
        o = res.results[c]["OUT"].astype(np.float64)
        outs.append(o[:, 0:64, :] / o[:, 64:65, :])
    outT = np.concatenate(outs, axis=0).reshape(HID, S)
    return (np.ascontiguousarray(outT.T.astype(np.float32)).reshape(1, S, HID),)



# revision 38
# speedup vs baseline: 1.2199x; 1.0648x over previous
"""Bass/Trainium2 kernel for BERT-style masked attention (B=1, S=4096, HID=1024, H=16).

Strategy: tensor-parallel over heads across 8 NeuronCores (2 heads/core).
Each core computes q/k/v projections for its 128 output columns from the
full (host-pretransposed) hidden states, runs masked softmax attention for
its 2 heads fully on-chip (flash-style, scores never hit DRAM), and writes
its [4096, 128] slice of the context. Host concatenates slices.

The key mask is key-only (same for every query/head), so masked key
positions are compacted away host-side: k/v projections and the attention
inner loop run only over the ~(S/2) surviving key positions.

Overlap structure: the k/v projection + v-transpose pipeline is interleaved
per 512-block with query-tile 0's attention, and each later query tile's
projection is drip-fed (one matmul per key chunk) through the preceding
tile's attention loop, so the PE and ACT engines stay dense end-to-end.
"""

import numpy as np
import ml_dtypes
from contextlib import ExitStack

import concourse.bass as bass
import concourse.tile as tile
from concourse import bacc, mybir
from concourse.bass_utils import run_bass_kernel_spmd
from concourse.masks import make_identity

f32 = mybir.dt.float32
f32r = mybir.dt.float32r
bf16 = mybir.dt.bfloat16
fp16 = mybir.dt.float16
f8 = mybir.dt.float8e4
i16 = mybir.dt.int16
AF = mybir.ActivationFunctionType
ALU = mybir.AluOpType
DR = mybir.MatmulPerfMode.DoubleRow

S = 4096
HID = 1024
D2 = 128          # per-core output columns (2 heads x 64)
NCH = HID // 128  # 8 hid chunks
NQT = S // 512    # 8 query tiles
SCALE = 64 ** -0.5
NEG = -1e30
# exp weights are computed as exp(s*SCALE - SHIFT); the shift cancels in
# the softmax normalization and keeps fp8 et values inside e4m3 range
SHIFT = 3.0
# Schraudolph exp for DVE-offloaded chunks: bf16_bits = round(A*s + B)
EXPA = float(128 * np.log2(np.e) * SCALE)
EXPB = float(16256.0 - 5.25 - 128 * np.log2(np.e) * SHIFT)
N_DVE = 5         # of nkca key-chunks per query tile, exp'd on DVE not ACT


def _emit(nc, tc, aps, nkb, nkca, rem):
    """nkb: # 512-wide key blocks for k/v projections (SKP = 512*nkb).
    nkca: # 128-wide key chunks the attention loop visits (<= 4*nkb).
    rem: # valid keys in the last chunk (pad rows of v1 stay zero, so
    padded keys drop out of both the context and the softmax sum)."""
    XT, XTKV, WQ, WK, WV, BQ, BK, BV, OUT = aps
    dve_kcs = set()
    if nkca >= 8:
        dve_kcs = {1 + i * nkca // N_DVE for i in range(N_DVE)}
    skp = 512 * nkb
    with ExitStack() as top:
        const = top.enter_context(tc.tile_pool(name="const", bufs=1))
        big = top.enter_context(tc.tile_pool(name="big", bufs=1))

        ident = const.tile([128, 128], f32)
        make_identity(nc, ident)

        wq = const.tile([128, NCH, 128], bf16)
        wk = const.tile([128, NCH, 128], bf16)
        wv = const.tile([128, NCH, 128], bf16)
        nc.sync.dma_start(out=wk, in_=WK.rearrange("(c p) d -> p c d", p=128))
        nc.scalar.dma_start(out=wq, in_=WQ.rearrange("(c p) d -> p c d", p=128))
        nc.scalar.dma_start(out=wv, in_=WV.rearrange("(c p) d -> p c d", p=128))

        bq = const.tile([128, 1], f32)
        bk = const.tile([128, 1], f32)
        bv = const.tile([128, 1], f32)
        nc.gpsimd.dma_start(out=bq, in_=BQ.unsqueeze(1))
        nc.gpsimd.dma_start(out=bk, in_=BK.unsqueeze(1))
        nc.gpsimd.dma_start(out=bv, in_=BV.unsqueeze(1))

        qT = big.tile([128, S], bf16)     # [d2, s] queries (both heads stacked)
        kT = big.tile([128, skp], bf16)   # [d2, sk] keys (compacted)
        vT = big.tile([128, skp], f32)    # [d2, sk] values (pre-transpose)
        v1 = big.tile([128, 2, nkca, 65], bf16)  # [k, head, chunk, d|1]
        ones_bf = const.tile([128, 1], bf16)
        nc.vector.memset(ones_bf, 1.0)
        if rem == 128:
            nc.vector.tensor_copy(v1[:, 0, :, 64:65], ones_bf.to_broadcast((128, nkca, 1)))
            nc.vector.tensor_copy(v1[:, 1, :, 64:65], ones_bf.to_broadcast((128, nkca, 1)))
        else:
            # last chunk: zero pad rows (v cols + ones col) so padded keys
            # contribute nothing to ctx or the softmax denominator
            for h in range(2):
                if nkca > 1:
                    nc.vector.tensor_copy(
                        v1[:, h, 0:nkca - 1, 64:65],
                        ones_bf.to_broadcast((128, nkca - 1, 1)))
                nc.vector.memset(v1[:, h, nkca - 1, :], 0.0)
                nc.vector.tensor_copy(v1[0:rem, h, nkca - 1, 64:65],
                                      ones_bf[0:rem])

        h0 = slice(0, 64)
        h1 = slice(64, 128)
        # can the next q tile's projection be drip-fed through the kc loop?
        drip = nkca >= NCH + 2
        d0 = nkca - NCH - 1  # chunk index at which the drip starts

        with tc.tile_pool(name="xwkp", bufs=3) as xwkp, \
             tc.tile_pool(name="xwp", bufs=3) as xwp, \
             tc.tile_pool(name="pkv", bufs=1, space="PSUM") as pkv, \
             tc.tile_pool(name="ppq", bufs=1, space="PSUM") as ppq, \
             tc.tile_pool(name="pss", bufs=2, space="PSUM") as pss, \
             tc.tile_pool(name="psc", bufs=1, space="PSUM") as psc, \
             tc.tile_pool(name="ep", bufs=4) as ep, \
             tc.tile_pool(name="cp", bufs=4) as cp:

            qstate = {}

            def qproj_dma(qt):
                qsl = slice(qt * 512, (qt + 1) * 512)
                xw = xwp.tile([128, NCH, 512], bf16, tag="xw", name=f"xw{qt}")
                for c in range(NCH):
                    nc.gpsimd.dma_start(
                        out=xw[:, c, :],
                        in_=XT[c * 128:(c + 1) * 128, qsl])
                pq = ppq.tile([128, 512], f32, tag="pqq", name=f"pq{qt}")
                qstate[qt] = (xw, pq)

            def qproj_mm(qt, c):
                xw, pq = qstate[qt]
                nc.tensor.matmul(pq, wq[:, c, :], xw[:, c, :],
                                 start=(c == 0), stop=(c == NCH - 1),
                                 skip_group_check=True)
                if c == NCH - 1:
                    qsl = slice(qt * 512, (qt + 1) * 512)
                    nc.vector.tensor_scalar_add(qT[:, qsl], pq, bq)

            def k_block(kb, xw=None):
                sl = slice(kb * 512, (kb + 1) * 512)
                if xw is None:
                    xw = xwkp.tile([128, NCH, 512], bf16, tag="xwk",
                                   name=f"xwk{kb}")
                    for c in range(NCH):
                        nc.sync.dma_start(
                            out=xw[:, c, :],
                            in_=XTKV[c * 128:(c + 1) * 128, sl])
                pk = pkv.tile([128, 512], f32, tag="pkv", name=f"pk{kb}")
                for c in range(NCH):
                    nc.tensor.matmul(pk, wk[:, c, :], xw[:, c, :],
                                     start=(c == 0), stop=(c == NCH - 1))
                nc.vector.tensor_scalar_add(kT[:, sl], pk, bk)
                return xw

            def v_block(kb, xw):
                sl = slice(kb * 512, (kb + 1) * 512)
                pv = pkv.tile([128, 512], f32, tag="pkv", name=f"pv{kb}")
                for c in range(NCH):
                    nc.tensor.matmul(pv, wv[:, c, :], xw[:, c, :],
                                     start=(c == 0), stop=(c == NCH - 1))
                nc.vector.tensor_scalar_add(vT[:, sl], pv, bv)

            def vt_chunk(kc):
                pt = pss.tile([128, 2, 512], f32, tag="ss", name=f"vt{kc}")
                nc.tensor.transpose(pt[:, 0, 0:128],
                                    vT[:, kc * 128:(kc + 1) * 128], ident)
                rows = 128 if kc < nkca - 1 else rem
                nc.vector.tensor_copy(v1[0:rows, 0, kc, 0:64],
                                      pt[0:rows, 0, 0:64])
                nc.vector.tensor_copy(v1[0:rows, 1, kc, 0:64],
                                      pt[0:rows, 0, 64:128])

            att = {}

            def att_begin(qt):
                if not drip or qt == 0:
                    qproj_dma(qt)
                    for c in range(NCH):
                        qproj_mm(qt, c)
                pc0 = psc.tile([65, 512], f32, tag="pc0", name=f"pc0_{qt}")
                pc1 = psc.tile([65, 512], f32, tag="pc1", name=f"pc1_{qt}")
                att[qt] = [pc0, pc1, []]

            def att_chunk(qt, kc):
                st = att[qt]
                qsl = slice(qt * 512, (qt + 1) * 512)
                ksl = slice(kc * 128, (kc + 1) * 128)
                ss = pss.tile([128, 2, 512], f32, tag="ss", name=f"ss{qt}_{kc}")
                nc.tensor.matmul(ss[:, 0, :], kT[h0, ksl], qT[h0, qsl],
                                 start=True, stop=True)
                nc.tensor.matmul(ss[:, 1, :], kT[h1, ksl], qT[h1, qsl],
                                 start=True, stop=True)
                if kc in dve_kcs:
                    eti = ep.tile([128, 2, 512], i16, tag="eti",
                                  name=f"eti{qt}_{kc}")
                    nc.vector.tensor_scalar(
                        out=eti, in0=ss, scalar1=EXPA, scalar2=EXPB,
                        op0=ALU.mult, op1=ALU.add)
                    et = eti.bitcast(bf16)
                else:
                    et = ep.tile([128, 2, 512], bf16, tag="et",
                                 name=f"et{qt}_{kc}")
                    nc.scalar.activation(et, ss, AF.Exp, scale=SCALE)
                st[2].append((kc, et))
                if len(st[2]) > 2:
                    # run ctx two chunks behind exp so the exp engines have
                    # two full periods of latency slack before PE needs et
                    pkc, pet = st[2].pop(0)
                    nc.tensor.matmul(st[0], v1[:, 0, pkc, :], pet[:, 0, :],
                                     start=(pkc == 0), stop=False)
                    nc.tensor.matmul(st[1], v1[:, 1, pkc, :], pet[:, 1, :],
                                     start=(pkc == 0), stop=False)
                if drip and qt + 1 < NQT:
                    if kc == min(2, d0):
                        qproj_dma(qt + 1)
                    elif d0 + 1 <= kc <= d0 + NCH:
                        qproj_mm(qt + 1, kc - d0 - 1)

            def att_flush(qt):
                pc0, pc1, pend = att[qt]
                while pend:
                    pkc, pet = pend.pop(0)
                    last = not pend
                    nc.tensor.matmul(pc0, v1[:, 0, pkc, :], pet[:, 0, :],
                                     start=(pkc == 0), stop=last)
                    nc.tensor.matmul(pc1, v1[:, 1, pkc, :], pet[:, 1, :],
                                     start=(pkc == 0), stop=last)
                att_drain(qt)

            def att_drain(qt):
                # free the PSUM accumulators right away: one copy per head
                # grabs ctx rows 0..63 plus the l row (64); host divides.
                for h in range(2):
                    pc = att[qt][h]
                    lc = cp.tile([65, 512], f32, tag="lc",
                                 name=f"lc{qt}_{h}")
                    nc.vector.tensor_copy(lc, pc)
                    nc.sync.dma_start(
                        out=OUT[h, :, qt * 512:(qt + 1) * 512], in_=lc)

            # ---- qt 0: interleaved with k/v projection blocks.
            # Critical-path DMAs first: qt0's x chunks and kv block 0's x
            # chunks alternate on the queue so both projection chains start
            # as early as possible.
            qsl0 = slice(0, 512)
            xw0 = xwp.tile([128, NCH, 512], bf16, tag="xw", name="xw0")
            xwk0 = xwkp.tile([128, NCH, 512], bf16, tag="xwk", name="xwk0")
            for c in range(NCH):
                nc.sync.dma_start(
                    out=xwk0[:, c, :],
                    in_=XTKV[c * 128:(c + 1) * 128, 0:512])
                nc.gpsimd.dma_start(
                    out=xw0[:, c, :],
                    in_=XT[c * 128:(c + 1) * 128, qsl0])
            pq0 = ppq.tile([128, 512], f32, tag="pqq", name="pq0")
            qstate[0] = (xw0, pq0)
            for c in range(NCH):
                qproj_mm(0, c)
            pc0_t = psc.tile([65, 512], f32, tag="pc0", name="pc0_0")
            pc1_t = psc.tile([65, 512], f32, tag="pc1", name="pc1_0")
            att[0] = [pc0_t, pc1_t, []]
            for kb in range(nkb):
                xw = k_block(kb, xw=xwk0 if kb == 0 else None)
                if kb * 4 < nkca:
                    att_chunk(0, kb * 4)  # ctx inside uses v1[kc-1] (ready)
                v_block(kb, xw)
                for j in range(4):
                    kc = kb * 4 + j
                    if kc >= nkca:
                        break
                    vt_chunk(kc)
                    if j > 0:
                        att_chunk(0, kc)
            att_flush(0)
            # ---- remaining q tiles
            for qt in range(1, NQT):
                att_begin(qt)
                for kc in range(nkca):
                    att_chunk(qt, kc)
                att_flush(qt)


_NC = {}


def _build(nkb, nkca, rem):
    key = (nkb, nkca, rem)
    if key in _NC:
        return _NC[key]
    nc = bacc.Bacc("TRN2", target_bir_lowering=False, debug=False)
    skp = 512 * nkb
    XT = nc.dram_tensor("XT", [HID, S], bf16, kind="ExternalInput").ap()
    XTKV = nc.dram_tensor("XTKV", [HID, skp], bf16, kind="ExternalInput").ap()
    WQ = nc.dram_tensor("WQ", [HID, D2], bf16, kind="ExternalInput").ap()
    WK = nc.dram_tensor("WK", [HID, D2], bf16, kind="ExternalInput").ap()
    WV = nc.dram_tensor("WV", [HID, D2], bf16, kind="ExternalInput").ap()
    BQ = nc.dram_tensor("BQ", [D2], f32, kind="ExternalInput").ap()
    BK = nc.dram_tensor("BK", [D2], f32, kind="ExternalInput").ap()
    BV = nc.dram_tensor("BV", [D2], f32, kind="ExternalInput").ap()
    OUT = nc.dram_tensor("OUT", [2, 65, S], f32, kind="ExternalOutput").ap()
    with tile.TileContext(nc) as tc:
        _emit(nc, tc, (XT, XTKV, WQ, WK, WV, BQ, BK, BV, OUT),
              nkb, nkca, rem)
    nc.compile()
    _NC[key] = nc
    return nc


def make_in_maps(hidden_states, attention_mask, Wq, bq, Wk, bk, Wv, bv):
    bfl = ml_dtypes.bfloat16
    x = np.asarray(hidden_states, dtype=np.float32).reshape(S, HID)
    xT = np.ascontiguousarray(x.T.astype(bfl))
    mask = np.asarray(attention_mask).reshape(S).astype(bool)
    idx = np.nonzero(mask)[0]
    m = len(idx)
    nkca = max(1, (m + 127) // 128)
    nkb = max(1, (nkca * 128 + 511) // 512)
    skp = nkb * 512
    # pad with position 0 (values are finite; pad slots masked to -inf below)
    idx_p = np.zeros(skp, np.int64)
    idx_p[:m] = idx
    xTkv = np.ascontiguousarray(xT[:, idx_p])
    rem = m - 128 * (nkca - 1)
    Wq = np.asarray(Wq, np.float32).astype(bfl)
    Wk = np.asarray(Wk, np.float32).astype(bfl)
    Wv = np.asarray(Wv, np.float32).astype(bfl)
    bq = np.asarray(bq, np.float32)
    bk = np.asarray(bk, np.float32)
    bv = np.asarray(bv, np.float32)
    in_maps = []
    for c in range(8):
        sl = slice(D2 * c, D2 * (c + 1))
        in_maps.append({
            "XT": xT, "XTKV": xTkv,
            "WQ": np.ascontiguousarray(Wq[:, sl]),
            "WK": np.ascontiguousarray(Wk[:, sl]),
            "WV": np.ascontiguousarray(Wv[:, sl]),
            "BQ": np.ascontiguousarray(bq[sl]),
            "BK": np.ascontiguousarray(bk[sl]),
            "BV": np.ascontiguousarray(bv[sl]),
        })
    return in_maps, nkb, nkca, rem


def kernel(hidden_states, attention_mask, Wq, bq, Wk, bk, Wv, bv):
    in_maps, nkb, nkca, rem = make_in_maps(
        hidden_states, attention_mask, Wq, bq, Wk, bk, Wv, bv)
    nc = _build(nkb, nkca, rem)
    res = run_bass_kernel_spmd(nc, in_maps, list(range(8)))
    # OUT[c]: [2, 65, S]; rows 0..63 = unnormalized ctx (transposed),
    # row 64 = softmax denominator l. Normalize + untranspose on host.
    outs = []
    for c in range(8):
        o = res.results[c]["OUT"].astype(np.float64)
        outs.append(o[:, 0:64, :] / o[:, 64:65, :])
    outT = np.concatenate(outs, axis=0).reshape(HID, S)
    return (np.ascontiguousarray(outT.T.astype(np.float32)).reshape(1, S, HID),)

